# revision 1
# baseline (speedup 1.0000x reference)
"""Trainium2 Bass kernel for nn_NonLocalBlock1D_new_position_multi_head.

Reference computation (B=8, C=512, T=2048, INTER=256, L=2):
  x = x + sinusoidal_PE(C, T)
  x1 = relu(w_tr @ x + b_tr)                          # 1x1 conv over channels
  temps = [dilated_tconv(x1, w_tc[l], d=l+1) for l in (0,1)] + [x1]
  per branch i (3 branches):
    g/th/ph = 1x1 convs of temps[i] -> (INTER, T)
    f  = softmax(th^T @ ph, axis=s)
    y_i = f @ g^T                                      # attention
  wy = w_W @ concat(y_i) + b_W
  out = batchnorm(wy, stats over (batch, time)) * gamma + beta + x1

Sharding: data-parallel over batch, one batch element per NeuronCore (8
cores).  The only cross-core communication is a [128,8] AllReduce for the
batch-norm statistics.

Per-core layout strategy:
  * channels on SBUF partitions in chunks of 128
  * attention computed transposed: S_T[s, t] = sum_i ph[i,s] th[i,t] so the
    second matmul (contracting over s, which is already the partition dim of
    both exp(S_T) and the transposed g projection) directly produces
    O_T[inter, t], which feeds the final W conv without any transposes.
  * softmax normalization deferred to the attention output: O_T gets
    multiplied by a broadcast of 1/rowsum; rowsum comes from an ones-lhsT
    matmul over exp(S_T), the partition-broadcast from a K=1 fp32 matmul.
  * matmuls run in float32r (full PE speed at N>=256); the exp(S) tensor,
    g-transpose and the dilated-conv temps are bf16.
  * biases: b_tr/b_W ride along free activation/tensor_scalar slots; b_g is
    folded into b_W on the host (softmax rows sum to one); b_th/b_ph are
    zero for this problem's inputs (asserted at build time -> plain copies).
"""

import os
import sys

sys.path.insert(0, "/opt/trn_rl_repo")
os.environ.setdefault("JAX_PLATFORMS", "")

import ml_dtypes
import numpy as np

import concourse.bass as bass  # noqa: F401  (kept for interactive debugging)
import concourse.mybir as mybir
import concourse.tile as tile
from concourse import bacc
from concourse import bass_utils
from concourse.bass import ts

F32 = mybir.dt.float32
F32R = mybir.dt.float32r
BF16 = mybir.dt.bfloat16
AF = mybir.ActivationFunctionType
ALU = mybir.AluOpType

B, C, T = 8, 512, 2048
INTER = C // 2
L = 2
NBR = L + 1          # 3 branches
P = 128
KO = C // P          # 4 channel chunks
KI = INTER // P      # 2 inter chunks
TB = 512             # time block
NTB = T // TB        # 4
SC = T // P          # 16 s-chunks of 128
N_CORES = 8
EPS = 1e-5

TEMPS_BF16 = True    # dilated-conv temps + their projection weights in bf16
ATT_BF16 = True      # exp(S) and g-transpose in bf16
DO_COLLECTIVE = os.environ.get("KERNEL_NOCOLL", "0") != "1"
PHASES = os.environ.get("KERNEL_PHASES", "ABDE")  # subset of "ABDE" for bisection


def _pos_encoding_np(c, t):
    pos = np.arange(t, dtype=np.float32)[:, None]
    i = np.arange(0, c, 2, dtype=np.float32)
    div = np.exp(-(np.log(10000.0) / c) * i).astype(np.float32)
    pe = np.zeros((t, c), dtype=np.float32)
    pe[:, 0::2] = np.sin(pos * div)
    pe[:, 1::2] = np.cos(pos * div)
    return np.ascontiguousarray(pe.T)  # (c, t)


def build_program(bias_thph_nonzero):
    """Builds the SPMD bass program (identical on all 8 cores)."""
    nc = bacc.Bacc("TRN2", target_bir_lowering=False, debug=False,
                   num_devices=N_CORES)

    TDT = BF16 if TEMPS_BF16 else F32R
    ADT = BF16 if ATT_BF16 else F32R

    # ---- DRAM I/O ----------------------------------------------------------
    x_d = nc.dram_tensor("x", [C, T], F32, kind="ExternalInput")
    pe_d = nc.dram_tensor("pe", [C, T], F32, kind="ExternalInput")
    w_trT_d = nc.dram_tensor("w_trT", [C, C], F32R, kind="ExternalInput")
    b_tr_d = nc.dram_tensor("b_tr", [C], F32, kind="ExternalInput")
    w_tcT_d = nc.dram_tensor("w_tcT", [L, 3, C, C], F32R, kind="ExternalInput")
    w_pT_r_d = nc.dram_tensor("w_pT_r", [NBR, 3, C, INTER], F32R,
                              kind="ExternalInput")
    w_pT_b_d = nc.dram_tensor("w_pT_b", [NBR, 3, C, INTER], TDT,
                              kind="ExternalInput")
    b_th_d = nc.dram_tensor("b_th", [NBR, INTER], F32, kind="ExternalInput")
    b_ph_d = nc.dram_tensor("b_ph", [NBR, INTER], F32, kind="ExternalInput")
    w_WT_d = nc.dram_tensor("w_WT", [NBR * INTER, C], F32R,
                            kind="ExternalInput")
    b_W_d = nc.dram_tensor("b_W", [C], F32, kind="ExternalInput")
    gamma_d = nc.dram_tensor("gamma", [C], F32, kind="ExternalInput")
    beta_d = nc.dram_tensor("beta", [C], F32, kind="ExternalInput")
    out_d = nc.dram_tensor("out", [C, T], F32, kind="ExternalOutput")

    aps = dict(
        x_r=x_d.ap().rearrange("(ko p) t -> p ko t", p=P),
        pe_r=pe_d.ap().rearrange("(ko p) t -> p ko t", p=P),
        w_trT_r=w_trT_d.ap().rearrange("(ko p) o -> p ko o", p=P),
        w_tcT_r=w_tcT_d.ap().rearrange("l k (ko p) o -> p l k ko o", p=P),
        w_pT_r_r=w_pT_r_d.ap().rearrange("b k (ko p) i -> p b k ko i", p=P),
        w_pT_b_r=w_pT_b_d.ap().rearrange("b k (ko p) i -> p b k ko i", p=P),
        b_th_r=b_th_d.ap().rearrange("m (ki p) -> p m ki", p=P),
        b_ph_r=b_ph_d.ap().rearrange("m (ki p) -> p m ki", p=P),
        w_WT_r=w_WT_d.ap().rearrange("(jo p) o -> p jo o", p=P),
        b_tr_r=b_tr_d.ap().rearrange("(ko p) -> p ko", p=P),
        b_W_r=b_W_d.ap().rearrange("(ko p) -> p ko", p=P),
        gamma_r=gamma_d.ap().rearrange("(ko p) -> p ko", p=P),
        beta_r=beta_d.ap().rearrange("(ko p) -> p ko", p=P),
        out_r=out_d.ap().rearrange("(ko p) t -> p ko t", p=P),
    )

    with tile.TileContext(nc) as tc:
        _emit(nc, tc, aps, bias_thph_nonzero)
    nc.compile()
    return nc


def _finish_early(nc, tc, aps, x1, wy, pool_w, pool_dram, pool_projw):
    # debug-only truncation: dump x1 so the program has an output
    nc.sync.dma_start(aps["out_r"][:, :, :], x1[:, :, 2:T + 2].bitcast(F32))
    if not pool_projw._released:
        pool_projw.release()
    pool_w.release()
    pool_dram.release()


def _emit(nc, tc, aps, bias_thph_nonzero):
    TDT = BF16 if TEMPS_BF16 else F32R
    ADT = BF16 if ATT_BF16 else F32R

    # ---- whole-kernel pools ------------------------------------------------
    pool_w = tc.alloc_tile_pool(name="whole", bufs=1)
    pool_projw = tc.alloc_tile_pool(name="projw", bufs=1)
    pool_dram = tc.alloc_tile_pool(name="drampool", bufs=1, space="DRAM")

    x1 = pool_w.tile([P, KO, T + 4], F32R, name="x1")  # 2 pad cols each side
    zpad = pool_w.tile([P, KO, 2], F32, name="zpad")
    wy = pool_w.tile([P, KO, T], F32, name="wy")
    w_WT_sb = pool_w.tile([P, NBR * KI, C], F32R, name="wWT")
    b_tr_sb = pool_w.tile([P, KO], F32, name="btr")
    b_W_sb = pool_w.tile([P, KO], F32, name="bW")
    gamma_sb = pool_w.tile([P, KO], F32, name="gammasb")
    beta_sb = pool_w.tile([P, KO], F32, name="betasb")
    b_th_sb = pool_w.tile([P, NBR, KI], F32, name="bth")
    b_ph_sb = pool_w.tile([P, NBR, KI], F32, name="bph")
    ones_col = pool_w.tile([P, 1], ADT, name="ones_col")    # lhsT for rowsum
    ones_f32c = pool_w.tile([P, 1], F32, name="ones_f32c")
    ones_row = pool_w.tile([1, P], F32, name="ones_row")    # lhsT for bcast
    stats = pool_w.tile([P, 8], F32, name="stats")
    sq_part = pool_w.tile([P, KO, NTB], F32, name="sq_part")
    sum_part = pool_w.tile([P, KO, NTB], F32, name="sum_part")
    eps_sb = pool_w.tile([P, 1], F32, name="eps_sb")
    nc.vector.memset(eps_sb[:], EPS)

    nc.sync.dma_start(b_tr_sb[:], aps["b_tr_r"])
    nc.vector.memset(ones_col[:], 1.0)
    nc.vector.memset(ones_f32c[:], 1.0)
    nc.vector.memset(ones_row[:], 1.0)
    nc.vector.memset(zpad[:], 0.0)
    nc.vector.tensor_copy(x1[:, :, 0:2], zpad[:])
    nc.vector.tensor_copy(x1[:, :, T + 2:T + 4], zpad[:])

    # ---- phase A: x + pe, w_tr conv, relu -> x1 ---------------------------
    with tc.tile_pool(name="phA", bufs=3) as pa, \
         tc.tile_pool(name="psA", bufs=3, space="PSUM") as psA, \
         tc.tile_pool(name="wtrp", bufs=1) as wtrp:
        w_trT_sb = wtrp.tile([P, KO, C], F32R, name="wtr")
        nc.sync.dma_start(w_trT_sb[:], aps["w_trT_r"])
        TA = 256
        for ta in range(T // TA):
            x_blk = pa.tile([P, KO, TA], F32, tag="xblk", name="xblk")
            pe_blk = pa.tile([P, KO, TA], F32, tag="peblk", name="peblk")
            xpe = pa.tile([P, KO, TA], F32R, tag="xpe", name="xpe")
            nc.sync.dma_start(x_blk[:], aps["x_r"][:, :, ts(ta, TA)])
            nc.sync.dma_start(pe_blk[:], aps["pe_r"][:, :, ts(ta, TA)])
            nc.vector.tensor_tensor(xpe[:], x_blk[:], pe_blk[:], ALU.add)
            for oc in range(KO):
                ps = psA.tile([P, TA], F32, tag="psA", name="psA")
                for kc in range(KO):
                    nc.tensor.matmul(ps[:], w_trT_sb[:, kc, ts(oc, P)],
                                     xpe[:, kc, :],
                                     start=(kc == 0), stop=(kc == KO - 1))
                nc.scalar.activation(x1[:, oc, 2 + ta * TA:2 + (ta + 1) * TA],
                                     ps[:], AF.Relu,
                                     bias=b_tr_sb[:, oc:oc + 1])

    # constants not needed until phases C/D/E -- emit after phase A so the
    # startup DMA bandwidth goes to x/pe
    nc.sync.dma_start(w_WT_sb[:], aps["w_WT_r"])
    nc.sync.dma_start(b_W_sb[:], aps["b_W_r"])
    nc.sync.dma_start(gamma_sb[:], aps["gamma_r"])
    nc.sync.dma_start(beta_sb[:], aps["beta_r"])
    nc.sync.dma_start(b_th_sb[:], aps["b_th_r"])
    nc.sync.dma_start(b_ph_sb[:], aps["b_ph_r"])

    if "B" not in PHASES:
        _finish_early(nc, tc, aps, x1, wy, pool_w, pool_dram, pool_projw)
        return

    # ---- phase B: dilated temporal convs -> t0, t1 ------------------------
    temps = [pool_w.tile([P, KO, T], TDT, name=f"temp{l}") for l in range(L)]
    with tc.tile_pool(name="wtc", bufs=2) as wtc, \
         tc.tile_pool(name="psB", bufs=3, space="PSUM") as psB:
        for l in range(L):
            d = l + 1
            w_tc_sb = wtc.tile([P, 3, KO, C], F32R, tag="wtc", name="wtcsb")
            nc.sync.dma_start(w_tc_sb[:], aps["w_tcT_r"][:, l])
            for tb in range(NTB):
                for oc in range(KO):
                    ps = psB.tile([P, TB], F32, tag="psB", name="psB")
                    first = True
                    for k in range(3):
                        off = 2 + tb * TB + (k - 1) * d
                        for kc in range(KO):
                            nc.tensor.matmul(
                                ps[:], w_tc_sb[:, k, kc, ts(oc, P)],
                                x1[:, kc, off:off + TB],
                                start=first, stop=(k == 2 and kc == KO - 1))
                            first = False
                    nc.scalar.copy(temps[l][:, oc, ts(tb, TB)], ps[:])

    if "D" not in PHASES:
        _finish_early(nc, tc, aps, x1, wy, pool_w, pool_dram, pool_projw)
        return

    # ---- phases C/D: per-branch projections + attention + W conv ----------
    pool_th = tc.alloc_tile_pool(name="thp", bufs=1)
    pool_ph = tc.alloc_tile_pool(name="php", bufs=1)
    pool_gx = tc.alloc_tile_pool(name="gxp", bufs=1)
    pool_pt = tc.alloc_tile_pool(name="ptp", bufs=1)
    pool_ot = tc.alloc_tile_pool(name="otp", bufs=2)
    pool_sm = tc.alloc_tile_pool(name="smp", bufs=2)
    pool_psD = tc.alloc_tile_pool(name="psD", bufs=1, space="PSUM")

    def branch(br):
        if br == L:  # tx = x1 (f32r, padded layout)
            def tx(kc, lo, hi):
                return x1[:, kc, 2 + lo:2 + hi]
            w_pT = aps["w_pT_r_r"]
            wdt = F32R
        else:
            def tx(kc, lo, hi, _t=temps[br]):
                return _t[:, kc, lo:hi]
            w_pT = aps["w_pT_b_r"]
            wdt = TDT

        wp = pool_projw.tile([P, 3, KO, INTER], wdt, tag=f"wp_{wdt}",
                             name=f"wp{br}")
        nc.sync.dma_start(wp[:], w_pT[:, br])
        # wp[:, 0]=g, wp[:, 1]=th, wp[:, 2]=ph

        th_sb = pool_th.tile([P, KI, T], F32R, tag="th", name="thsb")
        ph_sb = pool_ph.tile([P, KI, T], F32R, tag="ph", name="phsb")
        gx_sb = pool_gx.tile([P, SC, INTER], ADT, tag="gx", name="gxsb")

        # th/ph projections in [i, t] layout
        for kind, dst, bias_sb in ((1, th_sb, b_th_sb), (2, ph_sb, b_ph_sb)):
            for ic in range(KI):
                for tb in range(NTB):
                    ps = pool_psD.tile([P, TB], F32, tag="S", bufs=3,
                                       name="projps")
                    for kc in range(KO):
                        nc.tensor.matmul(ps[:], wp[:, kind, kc, ts(ic, P)],
                                         tx(kc, tb * TB, (tb + 1) * TB),
                                         start=(kc == 0), stop=(kc == KO - 1))
                    if bias_thph_nonzero:
                        nc.vector.tensor_scalar_add(
                            dst[:, ic, ts(tb, TB)], ps[:],
                            bias_sb[:, br, ic:ic + 1])
                    else:
                        # ACT copy: keeps the DVE free for the rowsum chain
                        nc.scalar.copy(dst[:, ic, ts(tb, TB)], ps[:])
        # g projection, transposed to [s, i] (b_g folded into b_W on host)
        for sc in range(SC):
            ps = pool_psD.tile([P, INTER], F32, tag="S", bufs=3, name="gxps")
            for kc in range(KO):
                nc.tensor.matmul(ps[:], tx(kc, sc * P, (sc + 1) * P),
                                 wp[:, 0, kc, :],
                                 start=(kc == 0), stop=(kc == KO - 1))
            nc.scalar.copy(gx_sb[:, sc, :], ps[:])

        # attention per time block
        for tb in range(NTB):
            p_t = pool_pt.tile([P, SC, TB], ADT, tag="pt", name="ptsb")
            for sc in range(SC):
                ps = pool_psD.tile([P, TB], F32, tag="S", bufs=3, name="Sps")
                for ic in range(KI):
                    nc.tensor.matmul(ps[:], ph_sb[:, ic, ts(sc, P)],
                                     th_sb[:, ic, ts(tb, TB)],
                                     start=(ic == 0), stop=(ic == KI - 1))
                nc.scalar.activation(p_t[:, sc, :], ps[:], AF.Exp)
            # rowsum over s: two independent DVE half-sums (first add per
            # half rides the bf16 2x mode), then one matmul contracts the
            # remaining partition dim
            hs = []
            for j in range(2):
                h = pool_sm.tile([P, TB], F32, tag=f"rsh{j}", bufs=1,
                                 name=f"rsh{j}")
                nc.vector.tensor_tensor(h[:], p_t[:, 8 * j, :],
                                        p_t[:, 8 * j + 1, :], ALU.add)
                for k in range(2, 8):
                    nc.vector.tensor_tensor(h[:], p_t[:, 8 * j + k, :], h[:],
                                            ALU.add)
                hs.append(h)
            nc.vector.tensor_tensor(hs[0][:], hs[1][:], hs[0][:], ALU.add)
            rs = pool_psD.tile([1, TB], F32, tag="rs", bufs=1, name="rsps")
            nc.tensor.matmul(rs[:], ones_f32c[:], hs[0][:], start=True,
                             stop=True)
            recip = pool_sm.tile([1, TB], F32, tag="recip", name="recipsb")
            nc.vector.reciprocal_approx_fast(out=recip[:], in_=rs[:])
            # broadcast recip across partitions (exact fp32 matmul, K=1)
            bc = pool_psD.tile([P, TB], F32, tag="bc", bufs=1, name="bcps")
            nc.tensor.matmul(bc[:], ones_row[:], recip[:], start=True,
                             stop=True)
            bc_sb = pool_sm.tile([P, TB], F32, tag="bcsb", name="bcsb")
            nc.scalar.copy(bc_sb[:], bc[:])
            # O_T = (gx_T.T @ P_T) * bcast  -> [i, t]
            o_sb = pool_ot.tile([P, KI, TB], F32R, tag="ot", name="otsb")
            for ic in range(KI):
                op = pool_psD.tile([P, TB], F32, tag="O", bufs=2, name="Ops")
                for sc in range(SC):
                    nc.tensor.matmul(op[:], gx_sb[:, sc, ts(ic, P)],
                                     p_t[:, sc, :],
                                     start=(sc == 0), stop=(sc == SC - 1))
                nc.vector.tensor_tensor(o_sb[:, ic, :], op[:], bc_sb[:],
                                        ALU.mult)
            # W-conv contribution of this branch
            for oc in range(KO):
                wps = pool_psD.tile([P, TB], F32, tag="W", bufs=1, name="Wps")
                for ic in range(KI):
                    nc.tensor.matmul(wps[:],
                                     w_WT_sb[:, br * KI + ic, ts(oc, P)],
                                     o_sb[:, ic, :],
                                     start=(ic == 0), stop=(ic == KI - 1))
                dst = wy[:, oc, ts(tb, TB)]
                if br == L:  # first branch emitted
                    nc.vector.tensor_scalar_add(dst, wps[:],
                                                b_W_sb[:, oc:oc + 1])
                else:
                    nc.vector.tensor_tensor(dst, wps[:], dst, ALU.add)
                if br == 1:  # last branch: fold BN partial stats in here
                    nc.vector.tensor_reduce(
                        sum_part[:, oc, tb:tb + 1], dst,
                        axis=mybir.AxisListType.X, op=ALU.add)
                    sq = pool_sm.tile([P, TB], F32, tag="sqsc", name="sqsc")
                    nc.scalar.activation(sq[:], dst, AF.Square,
                                         accum_out=sq_part[:, oc, tb:tb + 1])

    for br in (L, 0, 1):
        branch(br)
    for p in (pool_psD, pool_sm, pool_ot, pool_pt, pool_gx, pool_ph,
              pool_th, pool_projw):
        p.release()

    if "E" not in PHASES:
        _finish_early(nc, tc, aps, x1, wy, pool_w, pool_dram, pool_projw)
        return

    # ---- phase E: batch-norm stats + allreduce + finalize -----------------
    with tc.tile_pool(name="psE", bufs=2, space="PSUM") as psE, \
         tc.tile_pool(name="phE", bufs=3) as pheE, \
         tc.tile_pool(name="vecE", bufs=1) as vecE:
        nc.vector.tensor_reduce(stats[:, 0:4], sum_part[:],
                                axis=mybir.AxisListType.X, op=ALU.add)
        nc.vector.tensor_reduce(stats[:, 4:8], sq_part[:],
                                axis=mybir.AxisListType.X, op=ALU.add)

        allstats = vecE.tile([P, 8], F32, name="allstats")
        if DO_COLLECTIVE:
            bounce_in = pool_dram.tile([P, 8], F32, name="bouncein")
            bounce_out = pool_dram.tile([P, 8], F32, name="bounceout")
            nc.gpsimd.dma_start(bounce_in[:], stats[:])
            nc.gpsimd.collective_compute(
                "AllReduce", ALU.add,
                replica_groups=[list(range(N_CORES))],
                ins=[bounce_in.opt()],
                outs=[bounce_out.opt()],
            )
            nc.gpsimd.dma_start(allstats[:], bounce_out[:])
        else:
            nc.vector.tensor_copy(allstats[:], stats[:])

        inv_n = 1.0 / float(B * T) if DO_COLLECTIVE else 1.0 / float(T)
        mean = vecE.tile([P, KO], F32, name="meansb")
        var = vecE.tile([P, KO], F32, name="varsb")
        scale = vecE.tile([P, KO], F32, name="scalesb")
        shift = vecE.tile([P, KO], F32, name="shiftsb")
        tmp = vecE.tile([P, KO], F32, name="tmpsb")
        nc.vector.tensor_scalar_mul(mean[:], allstats[:, 0:4], inv_n)
        nc.vector.tensor_scalar_mul(var[:], allstats[:, 4:8], inv_n)
        nc.vector.tensor_tensor(tmp[:], mean[:], mean[:], ALU.mult)
        nc.vector.tensor_tensor(var[:], var[:], tmp[:], ALU.subtract)
        nc.scalar.activation(tmp[:], var[:], AF.Sqrt, bias=eps_sb[:])
        nc.vector.reciprocal(scale[:], tmp[:])
        nc.vector.tensor_tensor(scale[:], scale[:], gamma_sb[:], ALU.mult)
        nc.vector.tensor_tensor(tmp[:], mean[:], scale[:], ALU.mult)
        nc.vector.tensor_tensor(shift[:], beta_sb[:], tmp[:], ALU.subtract)

        for oc in range(KO):
            for tb in range(NTB):
                o_t = pheE.tile([P, TB], F32, tag="oute", name="oute")
                nc.vector.tensor_scalar(o_t[:], wy[:, oc, ts(tb, TB)],
                                        scale[:, oc:oc + 1],
                                        shift[:, oc:oc + 1],
                                        ALU.mult, ALU.add)
                nc.vector.tensor_tensor(
                    o_t[:], o_t[:],
                    x1[:, oc, 2 + tb * TB:2 + (tb + 1) * TB], ALU.add)
                nc.sync.dma_start(aps["out_r"][:, oc, ts(tb, TB)], o_t[:])

    pool_w.release()
    pool_dram.release()


_PROGRAM_CACHE = {}


def kernel(x, w_tr, b_tr, w_tc, w_g, b_g, w_th, b_th, w_ph, b_ph,
           w_W, b_W, gamma, beta):
    x = np.asarray(x, dtype=np.float32)
    w_tr = np.asarray(w_tr, dtype=np.float32)
    b_tr = np.asarray(b_tr, dtype=np.float32)
    w_tc = np.asarray(w_tc, dtype=np.float32)
    w_g = np.asarray(w_g, dtype=np.float32)
    b_g = np.asarray(b_g, dtype=np.float32)
    w_th = np.asarray(w_th, dtype=np.float32)
    b_th = np.asarray(b_th, dtype=np.float32)
    w_ph = np.asarray(w_ph, dtype=np.float32)
    b_ph = np.asarray(b_ph, dtype=np.float32)
    w_W = np.asarray(w_W, dtype=np.float32)
    b_W = np.asarray(b_W, dtype=np.float32)
    gamma = np.asarray(gamma, dtype=np.float32)
    beta = np.asarray(beta, dtype=np.float32)

    pe = _pos_encoding_np(C, T)

    w_trT = np.ascontiguousarray(w_tr.T)                       # (c, o)
    w_tcT = np.ascontiguousarray(w_tc.transpose(0, 2, 3, 1))   # (L, 3, c, o)
    # (NBR, kind{g,th,ph}, c, i)
    w_pT = np.ascontiguousarray(np.stack([
        np.stack([w_g[m].T, w_th[m].T, w_ph[m].T]) for m in range(NBR)
    ]))
    w_WT = np.ascontiguousarray(w_W.T)                         # (j, o)
    # fold g bias through the row-stochastic attention into b_W
    b_W_eff = b_W.copy()
    for m in range(NBR):
        b_W_eff += w_W[:, m * INTER:(m + 1) * INTER] @ b_g[m]

    bias_thph_nonzero = bool(np.abs(b_th).max() > 0 or np.abs(b_ph).max() > 0)

    key = (TEMPS_BF16, ATT_BF16, bias_thph_nonzero, DO_COLLECTIVE, PHASES)
    if key not in _PROGRAM_CACHE:
        _PROGRAM_CACHE[key] = build_program(bias_thph_nonzero)
    nc = _PROGRAM_CACHE[key]

    tdt_np = ml_dtypes.bfloat16 if TEMPS_BF16 else np.float32

    in_maps = []
    for c in range(N_CORES):
        in_maps.append({
            "x": x[c],
            "pe": pe,
            "w_trT": w_trT,
            "b_tr": b_tr,
            "w_tcT": w_tcT,
            "w_pT_r": w_pT,
            "w_pT_b": np.ascontiguousarray(w_pT.astype(tdt_np)),
            "b_th": b_th,
            "b_ph": b_ph,
            "w_WT": w_WT,
            "b_W": b_W_eff,
            "gamma": gamma,
            "beta": beta,
        })

    res = bass_utils.run_bass_kernel_spmd(
        nc, in_maps, core_ids=list(range(N_CORES)),
        trace=bool(int(os.environ.get("KERNEL_TRACE", "0"))),
    )
    out = np.stack([res.results[c]["out"] for c in range(N_CORES)], axis=0)
    kernel.last_results = res
    return out

